# revision 24
# baseline (speedup 1.0000x reference)
"""Trainium2 Bass kernel for the dual-modality dense transformer block.

Problem (hardcoded shapes): B=8, L=1024, H=512, NH=8, HD=64.
  - 6 linear projections (q/k/v for img and txt streams)
  - 4 full attentions: (q_img,KV_img), (q_txt,KV_txt), (q_img,KV_txt), (q_txt,KV_img)
  - out_img/out_txt linears on the averaged contexts, concat + cat linear
  - attention pooling (nn.MultiheadAttention-style) + out_proj

Sharding: pure data-parallel over batch B=8 across the 8 NeuronCores (all
attentions and the pooling are batch-independent).

Device algorithm (per core, one batch element):
  - All activations are kept feature-major ("transposed", [H, L]) so no
    on-device transposes are needed anywhere; host pre-transposes the two
    input tensors and all weights (and pre-scales q-side weights by
    1/sqrt(HD)).
  - V tensors are produced in natural [L, H] orientation directly (the
    transposed input serves as the stationary matmul operand for that).
  - Attention scores are computed transposed ([j, i] = keys on partitions),
    so softmax-exp output tiles are directly the moving operand for both the
    PV matmul and the ones-matmul that computes softmax denominators.
    Scores stay within +-3.2 for this problem so exp without max-subtraction
    is numerically safe (verified against the fp32 reference).
  - Normalization: fast-reciprocal of the denominator row (single custom DVE
    op, with the reference's 0.5 averaging factor folded into a scaled
    denominator copy), partition-broadcast by DMA through a DRAM scratch row,
    and applied during the fp32 PSUM eviction on the vector engine — deferred
    one pipeline stage (lag-1) so the PE never stalls on it.
  - bf16 operands everywhere, fp32 PSUM accumulation. Measured accuracy vs
    the fp32 reference: ~2e-3 of output absmax.
"""

import numpy as np
import ml_dtypes

import concourse.bass as bass
import concourse.tile as tile
from concourse import bacc, mybir
from concourse.bass_utils import run_bass_kernel_spmd
from concourse.dve_ops import RECIP_APPROX_FAST_CONSTS, RECIPROCAL_APPROX_FAST

B, L, H, NH, HD = 8, 1024, 512, 8, 64
BF = mybir.dt.bfloat16
F32 = mybir.dt.float32
Exp = mybir.ActivationFunctionType.Exp
bf16 = ml_dtypes.bfloat16

N_CORES = 8


def _emit(tc, d):
    nc = tc.nc
    import contextlib

    ctx = contextlib.ExitStack()
    with ctx:
        const = ctx.enter_context(tc.tile_pool(name="const", bufs=1))
        acts = ctx.enter_context(tc.tile_pool(name="acts", bufs=1))
        spool = ctx.enter_context(tc.tile_pool(name="spool", bufs=2))
        opool = ctx.enter_context(tc.tile_pool(name="opool", bufs=1))
        expool = ctx.enter_context(tc.tile_pool(name="expool", bufs=16))
        small = ctx.enter_context(tc.tile_pool(name="small", bufs=2))
        dscr = ctx.enter_context(tc.tile_pool(name="dscr", bufs=2, space="DRAM"))
        pmm = ctx.enter_context(tc.tile_pool(name="pmm", bufs=2, space="PSUM"))
        pctx = ctx.enter_context(tc.tile_pool(name="pctx", bufs=2, space="PSUM"))

        # ---- constants / inputs into SBUF ----
        def load(name, p_chunks, free, dt=BF):
            # one DMA per chunk: spreads the transfer across DMA queues so
            # the full tensor lands ~p_chunks x sooner than a single DMA
            t = const.tile([128, p_chunks, free], dt, tag=name)
            src_r = d[name].rearrange("(c p) n -> p c n", p=128)
            for c in range(p_chunks):
                nc.sync.dma_start(out=t[:, c, :], in_=src_r[:, c, :])
            return t

        def load_act(name, p_chunks, free, tag):
            t = acts.tile([128, p_chunks, free], BF, tag=tag)
            src_r = d[name].rearrange("(c p) n -> p c n", p=128)
            for c in range(p_chunks):
                nc.sync.dma_start(out=t[:, c, :], in_=src_r[:, c, :])
            return t

        def load2d(name, p, free, dt):
            t = const.tile([p, free], dt, tag=name)
            nc.sync.dma_start(out=t, in_=d[name])
            return t

        xt = load_act("xT", 4, L, "xT")
        w_qim = load("w_qim", 4, H)
        b_qim = load2d("b_qim", 128, 4, F32)
        w_kim = load("w_kim", 4, H)
        b_kim = load2d("b_kim", 128, 4, F32)
        w_vim = load("w_vim", 4, H)
        r_vim = load2d("r_vim", 1, H, BF)
        tt = load_act("tT", 4, L, "tT")
        w_qtx = load("w_qtx", 4, H)
        b_qtx = load2d("b_qtx", 128, 4, F32)
        w_ktx = load("w_ktx", 4, H)
        b_ktx = load2d("b_ktx", 128, 4, F32)
        w_vtx = load("w_vtx", 4, H)
        r_vtx = load2d("r_vtx", 1, H, BF)
        w_oim = load("w_oim", 4, H)
        b_oim = load2d("b_oim", 128, 4, F32)
        w_otx = load("w_otx", 4, H)
        b_otx = load2d("b_otx", 128, 4, F32)
        w_cat = load("w_cat", 8, H)
        b_cat = load2d("b_cat", 128, 4, F32)
        w_ip = load("w_ip", 4, 3 * H)
        b_ipqk = load2d("b_ipqk", 128, 8, F32)
        w_op = load("w_op", 4, H)
        r_op = load2d("r_op", 1, H, BF)

        ones_row = const.tile([1, 128], BF, tag="ones_row")
        nc.vector.memset(ones_row, 1.0)
        ones_col = const.tile([128, 1], BF, tag="ones_col")
        nc.vector.memset(ones_col, 1.0)
        # ---- helpers ----
        def proj_T(dst, dst_off, src, nk, w, w_off, bias, bias_off):
            """feature-major linear: dst[:, dst_off+m, :] = (w.T @ src) + bias."""
            for m in range(4):
                ps = pmm.tile([128, 1024], F32, tag="mm")
                for n in range(2):
                    for k in range(nk):
                        nc.tensor.matmul(
                            ps[:, n * 512 : (n + 1) * 512],
                            w[:, k, w_off + m * 128 : w_off + (m + 1) * 128],
                            src[:, k, n * 512 : (n + 1) * 512],
                            start=(k == 0),
                            stop=(k == nk - 1),
                        )
                o = dst[:, dst_off + m, :]
                if bias is not None:
                    nc.vector.tensor_scalar_add(
                        o, ps, bias[:, bias_off + m : bias_off + m + 1]
                    )
                else:
                    nc.vector.tensor_copy(out=o, in_=ps)

        def proj_N(dst, src, w, w_off, brow):
            """natural-orientation linear into the ones-augmented V layout:
            dst [128, 8(lc), 8(head), 65]; cols 0:64 of each head-block get
            (src.T @ w + brow), col 64 stays 1.0 (set by a prior memset)."""
            for lc2 in range(4):
                ps = pmm.tile([128, 1024], F32, tag="mm")
                for h in range(2):
                    lc = lc2 * 2 + h
                    for k in range(4):
                        nc.tensor.matmul(
                            ps[:, h * 512 : (h + 1) * 512],
                            src[:, k, lc * 128 : (lc + 1) * 128],
                            w[:, k, w_off : w_off + 512],
                            start=(k == 0),
                            stop=(brow is None and k == 3),
                            skip_group_check=True,
                        )
                    if brow is not None:
                        nc.tensor.matmul(
                            ps[:, h * 512 : (h + 1) * 512],
                            ones_row, brow, start=False, stop=True,
                            skip_group_check=True,
                        )
                nc.vector.tensor_copy(
                    out=dst[:, lc2 * 2 : lc2 * 2 + 2, :, 0:64],
                    in_=ps.rearrange("p (a b) -> p a b", a=2),
                )

        # Normalization of a pair is deferred one pipeline stage (lag-1):
        # the reciprocal chain of pair p runs on DVE while the PE streams
        # pair p+1's scores, so the PE never stalls on it. `pending[0]`
        # holds the not-yet-emitted normalize closure.
        pending = [None]

        def flush():
            if pending[0] is not None:
                pending[0]()
                pending[0] = None

        def attention(qT, kT, vN, s_dst, first, scale, mid_hook=None):
            """One multi-head attention; accumulates normalized ctx' into s_dst.

            vN is ones-augmented [128, 8(jt), 8(head), 65]: the PV matmul with
            M=65 produces ctx' on psum partitions 0..63 and the softmax
            denominator (ones row dot exp) on partition 64 for free."""
            for ih in range(2):
                if ih == 1 and mid_hook is not None:
                    mid_hook()
                i0 = ih * 512
                for p in range(4):
                    # per-jt exp tiles: same total capacity as two whole-pair
                    # buffers, but slots recycle jt-by-jt so the next pair's
                    # scores/exp can start before this pair's PV finishes
                    def _mk_ex():
                        t = expool.tile([128, 1024], BF, tag="exp")
                        return t

                    ex = [_mk_ex() for _ in range(8)]
                    # scores (transposed), both heads into one 2-bank tile
                    for jt in range(8):
                        ps = pmm.tile([128, 1024], F32, tag="mm")
                        for hh in range(2):
                            nc.tensor.matmul(
                                ps[:, hh * 512 : (hh + 1) * 512],
                                kT[hh * 64 : (hh + 1) * 64, p, jt * 128 : (jt + 1) * 128],
                                qT[hh * 64 : (hh + 1) * 64, p, i0 : i0 + 512],
                                start=True,
                                stop=True,
                                tile_position=(hh * 64, 0),
                            )
                        nc.scalar.activation(ex[jt], ps, Exp)
                    # PV + denominators: [v | ones].T @ exp per head; both
                    # heads share one 2-bank psum tile so one reciprocal op
                    # covers both denominator rows.
                    cps = pctx.tile([128, 1024], F32, tag="ctx")
                    for jt in range(8):
                        for hh in range(2):
                            nc.tensor.matmul(
                                cps[0:65, hh * 512 : (hh + 1) * 512],
                                vN[:, jt, p * 2 + hh, :],
                                ex[jt][:, hh * 512 : (hh + 1) * 512],
                                start=(jt == 0),
                                stop=(jt == 7),
                            )
                    flush()

                    def normalize(cps=cps, p=p, i0=i0, first=first, scale=scale):
                        # scaled copy of both denominator rows to SBUF (the
                        # custom recip op's fp32 bit-trick seed reads garbage
                        # from PSUM directly); scale=2 folds the reference's
                        # (ctx_a + ctx_b) * 0.5 averaging into 1/(2*den)
                        den = small.tile([1, 1024], F32, tag="den")
                        nc.vector.tensor_scalar_mul(den, cps[64:65, :], scale)
                        rc = small.tile([1, 1024], BF, tag="rc")
                        cdve = RECIP_APPROX_FAST_CONSTS
                        nc.vector._custom_dve(
                            RECIPROCAL_APPROX_FAST, out=rc, in0=den,
                            s0=cdve["s0"], s1=cdve["s1"], imm2=cdve["imm2"],
                        )
                        # partition-broadcast of the recips via DMA through
                        # a DRAM scratch row (SBUF APs forbid stride-0
                        # partitions; DRAM APs allow it) - off PE and DVE
                        dr = dscr.tile([1, 1024], BF, tag="dr")
                        nc.sync.dma_start(out=dr, in_=rc)
                        bcs = small.tile([128, 512], BF, tag="bcs")
                        for hh in range(2):
                            sl = dr[0:1, hh * 512 : (hh + 1) * 512]
                            bsrc = bass.AP(tensor=sl.tensor, offset=sl.offset,
                                           ap=[[0, 64]] + [list(a) for a in sl.ap[1:]])
                            nc.sync.dma_start(out=bcs[hh * 64 : (hh + 1) * 64, :], in_=bsrc)
                        o = s_dst[:, p, i0 : i0 + 512]
                        if first:
                            nc.vector.tensor_mul(o[0:64, :], cps[0:64, 0:512], bcs[0:64, :])
                            nc.vector.tensor_mul(o[64:128, :], cps[0:64, 512:1024], bcs[64:128, :])
                        else:
                            tmp = small.tile([128, 512], BF, tag="tmp")
                            nc.vector.tensor_mul(tmp[0:64, :], cps[0:64, 0:512], bcs[0:64, :])
                            nc.vector.tensor_mul(tmp[64:128, :], cps[0:64, 512:1024], bcs[64:128, :])
                            nc.vector.tensor_add(o, o, tmp)

                    pending[0] = normalize

        # ---- the network ----
        q_im = acts.tile([128, 4, L], BF, tag="q_im")
        k_im = acts.tile([128, 4, L], BF, tag="k_im")
        v_im = acts.tile([128, 8, 8, 65], BF, tag="v_im")
        nc.vector.memset(v_im, 1.0)
        q_tx = acts.tile([128, 4, L], BF, tag="q_tx")
        k_tx = acts.tile([128, 4, L], BF, tag="k_tx")
        v_tx = acts.tile([128, 8, 8, 65], BF, tag="v_tx")
        nc.vector.memset(v_tx, 1.0)

        proj_T(q_im, 0, xt, 4, w_qim, 0, b_qim, 0)
        proj_T(k_im, 0, xt, 4, w_kim, 0, b_kim, 0)
        proj_N(v_im, xt, w_vim, 0, r_vim)

        s_img = spool.tile([128, 4, L], BF, tag="s")
        attention(q_im, k_im, v_im, s_img, True, 2.0)   # ctx_img

        proj_T(q_tx, 0, tt, 4, w_qtx, 0, b_qtx, 0)
        proj_T(k_tx, 0, tt, 4, w_ktx, 0, b_ktx, 0)
        proj_N(v_tx, tt, w_vtx, 0, r_vtx)

        attention(q_im, k_tx, v_tx, s_img, False, 2.0)  # ctx_it

        s_txt = spool.tile([128, 4, L], BF, tag="s")
        attention(q_tx, k_tx, v_tx, s_txt, True, 2.0)   # ctx_txt
        # out_img projection emitted here: its matmuls fill the PE gaps of
        # the ACT-bound A4 stream (A3's last normalize was flushed inside A2)
        cat_a = acts.tile([128, 4, L], BF, tag="xT")
        proj_T(cat_a, 0, s_img, 4, w_oim, 0, b_oim, 0)

        attention(q_tx, k_im, v_im, s_txt, False, 2.0)  # ctx_ti
        flush()
        cat_b = acts.tile([128, 4, L], BF, tag="tT")
        proj_T(cat_b, 0, s_txt, 4, w_otx, 0, b_otx, 0)

        out_t = opool.tile([128, 4, L], BF, tag="out")
        for m in range(4):
            ps = pmm.tile([128, 1024], F32, tag="mm")
            for n in range(2):
                for k in range(8):
                    srck = cat_a if k < 4 else cat_b
                    nc.tensor.matmul(
                        ps[:, n * 512 : (n + 1) * 512],
                        w_cat[:, k, m * 128 : (m + 1) * 128],
                        srck[:, k % 4, n * 512 : (n + 1) * 512],
                        start=(k == 0),
                        stop=(k == 7),
                    )
            nc.vector.tensor_scalar_add(out_t[:, m, :], ps, b_cat[:, m : m + 1])

        q_pl = acts.tile([128, 4, L], BF, tag="q_im")
        k_pl = acts.tile([128, 4, L], BF, tag="q_tx")
        v_pl = acts.tile([128, 8, 8, 65], BF, tag="v_im")
        nc.vector.memset(v_pl, 1.0)
        proj_T(q_pl, 0, out_t, 4, w_ip, 0, b_ipqk, 0)
        proj_T(k_pl, 0, out_t, 4, w_ip, 512, b_ipqk, 4)
        proj_N(v_pl, out_t, w_ip, 1024, None)

        ctx_p = spool.tile([128, 4, L], BF, tag="s")

        def emit_out_proj(lcs):
            # out_proj (natural orientation) + bias, streamed to DRAM
            for lc in lcs:
                ps = pmm.tile([128, 1024], F32, tag="mm")
                for k in range(4):
                    nc.tensor.matmul(
                        ps[:, 0:512],
                        ctx_p[:, k, lc * 128 : (lc + 1) * 128],
                        w_op[:, k, :],
                        start=(k == 0),
                        stop=False,
                        skip_group_check=True,
                    )
                nc.tensor.matmul(
                    ps[:, 0:512], ones_row, r_op, start=False, stop=True,
                    skip_group_check=True,
                )
                res = small.tile([128, 512], F32, tag="res")
                nc.vector.tensor_copy(out=res, in_=ps[:, 0:512])
                nc.sync.dma_start(out=d["out"][lc * 128 : (lc + 1) * 128, :], in_=res)

        def pool_mid():
            # ihalf 0 of the pooling attention is fully normalized here, so
            # the first half of out_proj can overlap ihalf 1
            flush()
            emit_out_proj(range(4))

        attention(q_pl, k_pl, v_pl, ctx_p, True, 1.0, mid_hook=pool_mid)
        flush()
        emit_out_proj(range(4, 8))


_PROGRAM = None


def _build_program():
    global _PROGRAM
    if _PROGRAM is not None:
        return _PROGRAM
    nc = bacc.Bacc("TRN2", target_bir_lowering=False, debug=False)
    d = {}

    def din(name, shape, dt):
        d[name] = nc.dram_tensor(name, list(shape), dt, kind="ExternalInput").ap()

    din("xT", (H, L), BF)
    din("tT", (H, L), BF)
    for n in ("w_qim", "w_kim", "w_vim", "w_qtx", "w_ktx", "w_vtx", "w_oim", "w_otx"):
        din(n, (H, H), BF)
    din("w_cat", (2 * H, H), BF)
    din("w_ip", (H, 3 * H), BF)
    din("w_op", (H, H), BF)
    for n in ("b_qim", "b_kim", "b_qtx", "b_ktx", "b_oim", "b_otx", "b_cat"):
        din(n, (128, 4), F32)
    din("b_ipqk", (128, 8), F32)
    for n in ("r_vim", "r_vtx", "r_op"):
        din(n, (1, H), BF)
    d["out"] = nc.dram_tensor("out", [L, H], F32, kind="ExternalOutput").ap()

    with tile.TileContext(nc) as tc:
        _emit(tc, d)
    nc.compile()
    _PROGRAM = nc
    return nc


def _host_prep(inputs):
    f = lambda x: np.asarray(x, np.float32)

    def wT(w, scale=None):
        w = f(w)
        if scale is not None:
            w = w * scale
        return np.ascontiguousarray(w.T).astype(bf16)

    def bcol(b, scale=None):
        b = f(b)
        if scale is not None:
            b = b * scale
        return np.ascontiguousarray(b.reshape(-1, 128).T.astype(np.float32))

    def brow(b):
        return f(b).astype(bf16).reshape(1, -1)

    s = 1.0 / np.sqrt(HD)
    ipw = f(inputs["in_proj_w"]).copy()
    ipw[0:H] *= s
    ipb = f(inputs["in_proj_b"]).copy()
    ipb[0:H] *= s

    shared = {
        "w_qim": wT(inputs["w_q_img"], s),
        "w_kim": wT(inputs["w_k_img"]),
        "w_vim": wT(inputs["w_v_img"]),
        "w_qtx": wT(inputs["w_q_txt"], s),
        "w_ktx": wT(inputs["w_k_txt"]),
        "w_vtx": wT(inputs["w_v_txt"]),
        "w_oim": wT(inputs["w_out_img"]),
        "w_otx": wT(inputs["w_out_txt"]),
        "w_cat": wT(inputs["w_cat"]),
        "w_ip": wT(ipw),
        "w_op": wT(inputs["out_proj_w"]),
        "b_qim": bcol(inputs["b_q_img"], s),
        "b_kim": bcol(inputs["b_k_img"]),
        "b_qtx": bcol(inputs["b_q_txt"], s),
        "b_ktx": bcol(inputs["b_k_txt"]),
        "b_oim": bcol(inputs["b_out_img"]),
        "b_otx": bcol(inputs["b_out_txt"]),
        "b_cat": bcol(inputs["b_cat"]),
        "b_ipqk": bcol(ipb[0 : 2 * H]),
        "r_vim": brow(inputs["b_v_img"]),
        "r_vtx": brow(inputs["b_v_txt"]),
        "r_op": brow(inputs["out_proj_b"]),
    }
    hs = f(inputs["hidden_states"])
    tx = f(inputs["text"])
    in_maps = []
    for c in range(N_CORES):
        m = dict(shared)
        m["xT"] = np.ascontiguousarray(hs[c].T).astype(bf16)
        m["tT"] = np.ascontiguousarray(tx[c].T).astype(bf16)
        in_maps.append(m)
    return in_maps


def kernel(**inputs):
    nc = _build_program()
    in_maps = _host_prep(inputs)
    res = run_bass_kernel_spmd(nc, in_maps, core_ids=list(range(N_CORES)))
    out = np.stack([res.results[c]["out"] for c in range(N_CORES)])
    return out.astype(np.float32)


# revision 25
# speedup vs baseline: 1.0552x; 1.0552x over previous
"""Trainium2 Bass kernel for the dual-modality dense transformer block.

Problem (hardcoded shapes): B=8, L=1024, H=512, NH=8, HD=64.
  - 6 linear projections (q/k/v for img and txt streams)
  - 4 full attentions: (q_img,KV_img), (q_txt,KV_txt), (q_img,KV_txt), (q_txt,KV_img)
  - out_img/out_txt linears on the averaged contexts, concat + cat linear
  - attention pooling (nn.MultiheadAttention-style) + out_proj

Sharding: pure data-parallel over batch B=8 across the 8 NeuronCores (all
attentions and the pooling are batch-independent).

Device algorithm (per core, one batch element):
  - All activations are kept feature-major ("transposed", [H, L]) so no
    on-device transposes are needed anywhere; host pre-transposes the two
    input tensors and all weights (and pre-scales q-side weights by
    1/sqrt(HD)).
  - V tensors are produced in natural [L, H] orientation directly (the
    transposed input serves as the stationary matmul operand for that).
  - Attention scores are computed transposed ([j, i] = keys on partitions),
    so softmax-exp output tiles are directly the moving operand for both the
    PV matmul and the ones-matmul that computes softmax denominators.
    Scores stay within +-3.2 for this problem so exp without max-subtraction
    is numerically safe (verified against the fp32 reference).
  - Normalization: fast-reciprocal of the denominator row (single custom DVE
    op, with the reference's 0.5 averaging factor folded into a scaled
    denominator copy), partition-broadcast by DMA through a DRAM scratch row,
    and applied during the fp32 PSUM eviction on the vector engine — deferred
    one pipeline stage (lag-1) so the PE never stalls on it.
  - bf16 operands everywhere, fp32 PSUM accumulation. Measured accuracy vs
    the fp32 reference: ~2e-3 of output absmax.
"""

import numpy as np
import ml_dtypes

import concourse.bass as bass
import concourse.tile as tile
from concourse import bacc, mybir
from concourse.bass_utils import run_bass_kernel_spmd
from concourse.dve_ops import RECIP_APPROX_FAST_CONSTS, RECIPROCAL_APPROX_FAST

B, L, H, NH, HD = 8, 1024, 512, 8, 64
BF = mybir.dt.bfloat16
F32 = mybir.dt.float32
Exp = mybir.ActivationFunctionType.Exp
bf16 = ml_dtypes.bfloat16

N_CORES = 8


def _emit(tc, d):
    nc = tc.nc
    import contextlib

    ctx = contextlib.ExitStack()
    with ctx:
        const = ctx.enter_context(tc.tile_pool(name="const", bufs=1))
        acts = ctx.enter_context(tc.tile_pool(name="acts", bufs=1))
        spool = ctx.enter_context(tc.tile_pool(name="spool", bufs=2))
        opool = ctx.enter_context(tc.tile_pool(name="opool", bufs=1))
        expool = ctx.enter_context(tc.tile_pool(name="expool", bufs=2))
        small = ctx.enter_context(tc.tile_pool(name="small", bufs=2))
        dscr = ctx.enter_context(tc.tile_pool(name="dscr", bufs=2, space="DRAM"))
        pmm = ctx.enter_context(tc.tile_pool(name="pmm", bufs=2, space="PSUM"))
        pctx = ctx.enter_context(tc.tile_pool(name="pctx", bufs=2, space="PSUM"))

        # ---- constants / inputs into SBUF ----
        def load(name, p_chunks, free, dt=BF):
            # one DMA per chunk: spreads the transfer across DMA queues so
            # the full tensor lands ~p_chunks x sooner than a single DMA
            t = const.tile([128, p_chunks, free], dt, tag=name)
            src_r = d[name].rearrange("(c p) n -> p c n", p=128)
            for c in range(p_chunks):
                nc.sync.dma_start(out=t[:, c, :], in_=src_r[:, c, :])
            return t

        def load_act(name, p_chunks, free, tag):
            t = acts.tile([128, p_chunks, free], BF, tag=tag)
            src_r = d[name].rearrange("(c p) n -> p c n", p=128)
            for c in range(p_chunks):
                nc.sync.dma_start(out=t[:, c, :], in_=src_r[:, c, :])
            return t

        def load2d(name, p, free, dt):
            t = const.tile([p, free], dt, tag=name)
            nc.sync.dma_start(out=t, in_=d[name])
            return t

        xt = load_act("xT", 4, L, "xT")
        w_qim = load("w_qim", 4, H)
        b_qim = load2d("b_qim", 128, 4, F32)
        w_kim = load("w_kim", 4, H)
        b_kim = load2d("b_kim", 128, 4, F32)
        w_vim = load("w_vim", 4, H)
        r_vim = load2d("r_vim", 1, H, BF)
        tt = load_act("tT", 4, L, "tT")
        w_qtx = load("w_qtx", 4, H)
        b_qtx = load2d("b_qtx", 128, 4, F32)
        w_ktx = load("w_ktx", 4, H)
        b_ktx = load2d("b_ktx", 128, 4, F32)
        w_vtx = load("w_vtx", 4, H)
        r_vtx = load2d("r_vtx", 1, H, BF)
        w_oim = load("w_oim", 4, H)
        b_oim = load2d("b_oim", 128, 4, F32)
        w_otx = load("w_otx", 4, H)
        b_otx = load2d("b_otx", 128, 4, F32)
        w_cat = load("w_cat", 8, H)
        b_cat = load2d("b_cat", 128, 4, F32)
        w_ip = load("w_ip", 4, 3 * H)
        b_ipqk = load2d("b_ipqk", 128, 8, F32)
        w_op = load("w_op", 4, H)
        r_op = load2d("r_op", 1, H, BF)

        ones_row = const.tile([1, 128], BF, tag="ones_row")
        nc.vector.memset(ones_row, 1.0)
        ones_col = const.tile([128, 1], BF, tag="ones_col")
        nc.vector.memset(ones_col, 1.0)
        # ---- helpers ----
        def proj_T(dst, dst_off, src, nk, w, w_off, bias, bias_off):
            """feature-major linear: dst[:, dst_off+m, :] = (w.T @ src) + bias."""
            for m in range(4):
                ps = pmm.tile([128, 1024], F32, tag="mm")
                for n in range(2):
                    for k in range(nk):
                        nc.tensor.matmul(
                            ps[:, n * 512 : (n + 1) * 512],
                            w[:, k, w_off + m * 128 : w_off + (m + 1) * 128],
                            src[:, k, n * 512 : (n + 1) * 512],
                            start=(k == 0),
                            stop=(k == nk - 1),
                        )
                o = dst[:, dst_off + m, :]
                if bias is not None:
                    nc.vector.tensor_scalar_add(
                        o, ps, bias[:, bias_off + m : bias_off + m + 1]
                    )
                else:
                    nc.vector.tensor_copy(out=o, in_=ps)

        def proj_N(dst, src, w, w_off, brow):
            """natural-orientation linear into the ones-augmented V layout:
            dst [128, 8(lc), 8(head), 65]; cols 0:64 of each head-block get
            (src.T @ w + brow), col 64 stays 1.0 (set by a prior memset)."""
            for lc2 in range(4):
                ps = pmm.tile([128, 1024], F32, tag="mm")
                for h in range(2):
                    lc = lc2 * 2 + h
                    for k in range(4):
                        nc.tensor.matmul(
                            ps[:, h * 512 : (h + 1) * 512],
                            src[:, k, lc * 128 : (lc + 1) * 128],
                            w[:, k, w_off : w_off + 512],
                            start=(k == 0),
                            stop=(brow is None and k == 3),
                            skip_group_check=True,
                        )
                    if brow is not None:
                        nc.tensor.matmul(
                            ps[:, h * 512 : (h + 1) * 512],
                            ones_row, brow, start=False, stop=True,
                            skip_group_check=True,
                        )
                nc.vector.tensor_copy(
                    out=dst[:, lc2 * 2 : lc2 * 2 + 2, :, 0:64],
                    in_=ps.rearrange("p (a b) -> p a b", a=2),
                )

        # Normalization of a pair is deferred one pipeline stage (lag-1):
        # the reciprocal chain of pair p runs on DVE while the PE streams
        # pair p+1's scores, so the PE never stalls on it. `pending[0]`
        # holds the not-yet-emitted normalize closure.
        pending = [None]

        def flush():
            if pending[0] is not None:
                pending[0]()
                pending[0] = None

        def attention(qT, kT, vN, s_dst, first, scale, mid_hook=None):
            """One multi-head attention; accumulates normalized ctx' into s_dst.

            vN is ones-augmented [128, 8(jt), 8(head), 65]: the PV matmul with
            M=65 produces ctx' on psum partitions 0..63 and the softmax
            denominator (ones row dot exp) on partition 64 for free."""
            for ih in range(2):
                if ih == 1 and mid_hook is not None:
                    mid_hook()
                i0 = ih * 512
                for p in range(4):
                    ex = expool.tile([128, 8, 1024], BF, tag="exp")
                    # scores (transposed), both heads into one 2-bank tile
                    for jt in range(8):
                        ps = pmm.tile([128, 1024], F32, tag="mm")
                        for hh in range(2):
                            nc.tensor.matmul(
                                ps[:, hh * 512 : (hh + 1) * 512],
                                kT[hh * 64 : (hh + 1) * 64, p, jt * 128 : (jt + 1) * 128],
                                qT[hh * 64 : (hh + 1) * 64, p, i0 : i0 + 512],
                                start=True,
                                stop=True,
                                tile_position=(hh * 64, 0),
                            )
                        nc.scalar.activation(ex[:, jt, :], ps, Exp)
                    # PV + denominators: [v | ones].T @ exp per head; both
                    # heads share one 2-bank psum tile so one reciprocal op
                    # covers both denominator rows.
                    cps = pctx.tile([128, 1024], F32, tag="ctx")
                    for jt in range(8):
                        for hh in range(2):
                            nc.tensor.matmul(
                                cps[0:65, hh * 512 : (hh + 1) * 512],
                                vN[:, jt, p * 2 + hh, :],
                                ex[:, jt, hh * 512 : (hh + 1) * 512],
                                start=(jt == 0),
                                stop=(jt == 7),
                            )
                    flush()

                    def normalize(cps=cps, p=p, i0=i0, first=first, scale=scale):
                        # scaled copy of both denominator rows to SBUF (the
                        # custom recip op's fp32 bit-trick seed reads garbage
                        # from PSUM directly); scale=2 folds the reference's
                        # (ctx_a + ctx_b) * 0.5 averaging into 1/(2*den)
                        den = small.tile([1, 1024], F32, tag="den")
                        nc.vector.tensor_scalar_mul(den, cps[64:65, :], scale)
                        rc = small.tile([1, 1024], BF, tag="rc")
                        cdve = RECIP_APPROX_FAST_CONSTS
                        nc.vector._custom_dve(
                            RECIPROCAL_APPROX_FAST, out=rc, in0=den,
                            s0=cdve["s0"], s1=cdve["s1"], imm2=cdve["imm2"],
                        )
                        # partition-broadcast of the recips via DMA through
                        # a DRAM scratch row (SBUF APs forbid stride-0
                        # partitions; DRAM APs allow it) - off PE and DVE
                        dr = dscr.tile([1, 1024], BF, tag="dr")
                        nc.sync.dma_start(out=dr, in_=rc)
                        bcs = small.tile([128, 512], BF, tag="bcs")
                        for hh in range(2):
                            sl = dr[0:1, hh * 512 : (hh + 1) * 512]
                            bsrc = bass.AP(tensor=sl.tensor, offset=sl.offset,
                                           ap=[[0, 64]] + [list(a) for a in sl.ap[1:]])
                            nc.sync.dma_start(out=bcs[hh * 64 : (hh + 1) * 64, :], in_=bsrc)
                        o = s_dst[:, p, i0 : i0 + 512]
                        if first:
                            nc.vector.tensor_mul(o[0:64, :], cps[0:64, 0:512], bcs[0:64, :])
                            nc.vector.tensor_mul(o[64:128, :], cps[0:64, 512:1024], bcs[64:128, :])
                        else:
                            tmp = small.tile([128, 512], BF, tag="tmp")
                            nc.vector.tensor_mul(tmp[0:64, :], cps[0:64, 0:512], bcs[0:64, :])
                            nc.vector.tensor_mul(tmp[64:128, :], cps[0:64, 512:1024], bcs[64:128, :])
                            nc.vector.tensor_add(o, o, tmp)

                    pending[0] = normalize

        # ---- the network ----
        q_im = acts.tile([128, 4, L], BF, tag="q_im")
        k_im = acts.tile([128, 4, L], BF, tag="k_im")
        v_im = acts.tile([128, 8, 8, 65], BF, tag="v_im")
        nc.vector.memset(v_im, 1.0)
        q_tx = acts.tile([128, 4, L], BF, tag="q_tx")
        k_tx = acts.tile([128, 4, L], BF, tag="k_tx")
        v_tx = acts.tile([128, 8, 8, 65], BF, tag="v_tx")
        nc.vector.memset(v_tx, 1.0)

        proj_T(q_im, 0, xt, 4, w_qim, 0, b_qim, 0)
        proj_T(k_im, 0, xt, 4, w_kim, 0, b_kim, 0)
        proj_N(v_im, xt, w_vim, 0, r_vim)

        s_img = spool.tile([128, 4, L], BF, tag="s")
        attention(q_im, k_im, v_im, s_img, True, 2.0)   # ctx_img

        proj_T(q_tx, 0, tt, 4, w_qtx, 0, b_qtx, 0)
        proj_T(k_tx, 0, tt, 4, w_ktx, 0, b_ktx, 0)
        proj_N(v_tx, tt, w_vtx, 0, r_vtx)

        attention(q_im, k_tx, v_tx, s_img, False, 2.0)  # ctx_it

        s_txt = spool.tile([128, 4, L], BF, tag="s")
        attention(q_tx, k_tx, v_tx, s_txt, True, 2.0)   # ctx_txt
        # out_img projection emitted here: its matmuls fill the PE gaps of
        # the ACT-bound A4 stream (A3's last normalize was flushed inside A2)
        cat_a = acts.tile([128, 4, L], BF, tag="xT")
        proj_T(cat_a, 0, s_img, 4, w_oim, 0, b_oim, 0)

        attention(q_tx, k_im, v_im, s_txt, False, 2.0)  # ctx_ti
        flush()
        cat_b = acts.tile([128, 4, L], BF, tag="tT")
        proj_T(cat_b, 0, s_txt, 4, w_otx, 0, b_otx, 0)

        out_t = opool.tile([128, 4, L], BF, tag="out")
        for m in range(4):
            ps = pmm.tile([128, 1024], F32, tag="mm")
            for n in range(2):
                for k in range(8):
                    srck = cat_a if k < 4 else cat_b
                    nc.tensor.matmul(
                        ps[:, n * 512 : (n + 1) * 512],
                        w_cat[:, k, m * 128 : (m + 1) * 128],
                        srck[:, k % 4, n * 512 : (n + 1) * 512],
                        start=(k == 0),
                        stop=(k == 7),
                    )
            nc.vector.tensor_scalar_add(out_t[:, m, :], ps, b_cat[:, m : m + 1])

        q_pl = acts.tile([128, 4, L], BF, tag="q_im")
        k_pl = acts.tile([128, 4, L], BF, tag="q_tx")
        v_pl = acts.tile([128, 8, 8, 65], BF, tag="v_im")
        nc.vector.memset(v_pl, 1.0)
        proj_T(q_pl, 0, out_t, 4, w_ip, 0, b_ipqk, 0)
        proj_T(k_pl, 0, out_t, 4, w_ip, 512, b_ipqk, 4)
        proj_N(v_pl, out_t, w_ip, 1024, None)

        ctx_p = spool.tile([128, 4, L], BF, tag="s")

        def emit_out_proj(lcs):
            # out_proj (natural orientation) + bias, streamed to DRAM
            for lc in lcs:
                ps = pmm.tile([128, 1024], F32, tag="mm")
                for k in range(4):
                    nc.tensor.matmul(
                        ps[:, 0:512],
                        ctx_p[:, k, lc * 128 : (lc + 1) * 128],
                        w_op[:, k, :],
                        start=(k == 0),
                        stop=False,
                        skip_group_check=True,
                    )
                nc.tensor.matmul(
                    ps[:, 0:512], ones_row, r_op, start=False, stop=True,
                    skip_group_check=True,
                )
                res = small.tile([128, 512], F32, tag="res")
                nc.vector.tensor_copy(out=res, in_=ps[:, 0:512])
                nc.sync.dma_start(out=d["out"][lc * 128 : (lc + 1) * 128, :], in_=res)

        def pool_mid():
            # ihalf 0 of the pooling attention is fully normalized here, so
            # the first half of out_proj can overlap ihalf 1
            flush()
            emit_out_proj(range(4))

        attention(q_pl, k_pl, v_pl, ctx_p, True, 1.0, mid_hook=pool_mid)
        flush()
        emit_out_proj(range(4, 8))


_PROGRAM = None


def _build_program():
    global _PROGRAM
    if _PROGRAM is not None:
        return _PROGRAM
    nc = bacc.Bacc("TRN2", target_bir_lowering=False, debug=False)
    d = {}

    def din(name, shape, dt):
        d[name] = nc.dram_tensor(name, list(shape), dt, kind="ExternalInput").ap()

    din("xT", (H, L), BF)
    din("tT", (H, L), BF)
    for n in ("w_qim", "w_kim", "w_vim", "w_qtx", "w_ktx", "w_vtx", "w_oim", "w_otx"):
        din(n, (H, H), BF)
    din("w_cat", (2 * H, H), BF)
    din("w_ip", (H, 3 * H), BF)
    din("w_op", (H, H), BF)
    for n in ("b_qim", "b_kim", "b_qtx", "b_ktx", "b_oim", "b_otx", "b_cat"):
        din(n, (128, 4), F32)
    din("b_ipqk", (128, 8), F32)
    for n in ("r_vim", "r_vtx", "r_op"):
        din(n, (1, H), BF)
    d["out"] = nc.dram_tensor("out", [L, H], F32, kind="ExternalOutput").ap()

    with tile.TileContext(nc) as tc:
        _emit(tc, d)
    nc.compile()
    _PROGRAM = nc
    return nc


def _host_prep(inputs):
    f = lambda x: np.asarray(x, np.float32)

    def wT(w, scale=None):
        w = f(w)
        if scale is not None:
            w = w * scale
        return np.ascontiguousarray(w.T).astype(bf16)

    def bcol(b, scale=None):
        b = f(b)
        if scale is not None:
            b = b * scale
        return np.ascontiguousarray(b.reshape(-1, 128).T.astype(np.float32))

    def brow(b):
        return f(b).astype(bf16).reshape(1, -1)

    s = 1.0 / np.sqrt(HD)
    ipw = f(inputs["in_proj_w"]).copy()
    ipw[0:H] *= s
    ipb = f(inputs["in_proj_b"]).copy()
    ipb[0:H] *= s

    shared = {
        "w_qim": wT(inputs["w_q_img"], s),
        "w_kim": wT(inputs["w_k_img"]),
        "w_vim": wT(inputs["w_v_img"]),
        "w_qtx": wT(inputs["w_q_txt"], s),
        "w_ktx": wT(inputs["w_k_txt"]),
        "w_vtx": wT(inputs["w_v_txt"]),
        "w_oim": wT(inputs["w_out_img"]),
        "w_otx": wT(inputs["w_out_txt"]),
        "w_cat": wT(inputs["w_cat"]),
        "w_ip": wT(ipw),
        "w_op": wT(inputs["out_proj_w"]),
        "b_qim": bcol(inputs["b_q_img"], s),
        "b_kim": bcol(inputs["b_k_img"]),
        "b_qtx": bcol(inputs["b_q_txt"], s),
        "b_ktx": bcol(inputs["b_k_txt"]),
        "b_oim": bcol(inputs["b_out_img"]),
        "b_otx": bcol(inputs["b_out_txt"]),
        "b_cat": bcol(inputs["b_cat"]),
        "b_ipqk": bcol(ipb[0 : 2 * H]),
        "r_vim": brow(inputs["b_v_img"]),
        "r_vtx": brow(inputs["b_v_txt"]),
        "r_op": brow(inputs["out_proj_b"]),
    }
    hs = f(inputs["hidden_states"])
    tx = f(inputs["text"])
    in_maps = []
    for c in range(N_CORES):
        m = dict(shared)
        m["xT"] = np.ascontiguousarray(hs[c].T).astype(bf16)
        m["tT"] = np.ascontiguousarray(tx[c].T).astype(bf16)
        in_maps.append(m)
    return in_maps


def kernel(**inputs):
    nc = _build_program()
    in_maps = _host_prep(inputs)
    res = run_bass_kernel_spmd(nc, in_maps, core_ids=list(range(N_CORES)))
    out = np.stack([res.results[c]["out"] for c in range(N_CORES)])
    return out.astype(np.float32)
